# revision 1
# baseline (speedup 1.0000x reference)
"""Trainium2 Bass kernel for nn_BatchFFTMA: 9216 independent 65x65 FFT-MA sims.

Math (validated in proto.py against the jax reference):
  For each window w (patch p = noise[r0:r0+65, c0:c0+65], angle theta):
    Wf' = Cpt^T p Cpt   with Cpt = F*diag((-1)^k)  (patch DFT; center-pixel
          phase e^{2pi i 32k/65} and the principal-sqrt half-shift phase
          combine to exactly (-1)^k)
    E   = Re(Cq^T R Cq) with Cq = F*diag(e^{-2pi i 33 k/65})  (DFT of the
          ifftshift-aligned R -> real, even spectrum; the reference's
          fftshift-vs-ifftshift off-by-one is the source of the half-shift)
    R   = exp(-sqrt(q)), q = alpha*x_r^2 + beta*x_c^2 + gamma*x_r*x_c
    gp  = sqrt(relu(E+1e-8)), gn = sqrt(relu(-(E+1e-8)))
    g+  = gp*SGP, g- = gn*SGN   (SGP/SGN: +-1 fields from sqrt branch cuts)
    a = Wf'_r*g+, b = Wf'_i*g-; Xr = a - b
    VC = sum(Xr); X00 = Xr[0,0]; S = sum((Wf'_r^2+Wf'_i^2)*|E+1e-8|)
    out_w = ((VC-X00)/N^2) / (sqrt((S-X00^2)/(N^2(N^2-1))) + 1e-6)
  (v = ifft2 never materialized: center pixel via phase fold, mean via X[0,0],
   std via Parseval.)

Sharding: window/batch axis across 8 cores (1152 windows each, 12 output rows).
Precision: patch DFT + spectral chain bf16 (validated), R field + R DFT fp32.
"""
import os
import numpy as np
import ml_dtypes

H, W, D = 96, 96, 32
N = 65
N2 = N * N
A_, B_ = 15.0, 3.0
NCORE = 8
WPC = H * W // NCORE      # 1152 windows per core
RPC = H // NCORE          # 12 output rows per core
CB = 18                   # windows per vector chunk
GRP = 3                   # windows per matmul/PSUM group
NGRP = CB // GRP

_bf16 = ml_dtypes.bfloat16


def _host_constants():
    k = np.arange(N)
    F = np.exp(-2j * np.pi * np.outer(k, k) / N)
    Cpt = F * ((-1.0) ** k)[None, :]
    Cq = F * np.exp(-2j * np.pi * k * 33 / N)[None, :]
    Cr = Cpt.real.astype(np.float32)
    Ci = Cpt.imag.astype(np.float32)
    Qr = Cq.real.astype(np.float32)
    Qi = Cq.imag.astype(np.float32)
    pconst = np.concatenate([Cr, Ci, -Ci, Cr], axis=1).astype(_bf16)   # [65, 260]
    rconst = np.concatenate([Qr, Qi, -Qi], axis=1).astype(np.float32)  # [65, 195]

    k1, k2 = np.meshgrid(k, k, indexing="ij")
    ksum = k1 + k2
    m = ksum % N
    extra = (-1.0) ** (ksum // N)
    SGP = extra * np.where(m <= 32, 1.0, -1.0)
    SGN = extra * np.where(m == 0, 1.0, -1.0)
    sgp_t = np.tile(SGP, (1, CB)).astype(_bf16)   # [65, 65*CB]
    sgn_t = np.tile(SGN, (1, CB)).astype(_bf16)
    sgconst = np.concatenate([sgp_t, sgn_t], axis=1)  # [65, 2*65*CB]
    return pconst, rconst, sgconst


def _build_program(nchunk):
    import concourse.bacc as bacc
    import concourse.mybir as mybir
    from concourse.tile import TileContext

    f32 = mybir.dt.float32
    bf16 = mybir.dt.bfloat16
    AF = mybir.ActivationFunctionType
    ALU = mybir.AluOpType
    AX = mybir.AxisListType

    nwin = nchunk * CB
    nc = bacc.Bacc()
    strips_in = nc.declare_dram_parameter("strips", [N, 12 * 160], bf16, isOutput=False)
    rhsq_in = nc.declare_dram_parameter("rhsq", [3, nwin * N], f32, isOutput=False)
    qbasis_in = nc.declare_dram_parameter("qbasis", [3, N], f32, isOutput=False)
    pconst_in = nc.declare_dram_parameter("pconst", [N, 260], bf16, isOutput=False)
    rconst_in = nc.declare_dram_parameter("rconst", [N, 195], f32, isOutput=False)
    sg_in = nc.declare_dram_parameter("sgconst", [N, 2 * N * CB], bf16, isOutput=False)
    out_d = nc.declare_dram_parameter("out", [131, nwin], f32, isOutput=True)

    with TileContext(nc) as tc:
        with (
            tc.tile_pool(name="const", bufs=1) as cpool,
            tc.tile_pool(name="qsb", bufs=8) as qpool,
            tc.tile_pool(name="rsb", bufs=2) as rpool,
            tc.tile_pool(name="o1p", bufs=2) as o1ppool,
            tc.tile_pool(name="o1r", bufs=2) as o1rpool,
            tc.tile_pool(name="wfsb", bufs=2) as wfpool,
            tc.tile_pool(name="esb", bufs=2) as epool,
            tc.tile_pool(name="spec", bufs=2) as spool,
            tc.tile_pool(name="psq", bufs=2, space="PSUM") as pq_,
            tc.tile_pool(name="ps1p", bufs=2, space="PSUM") as pp1,
            tc.tile_pool(name="ps1r", bufs=2, space="PSUM") as pr1,
            tc.tile_pool(name="pswf", bufs=1, space="PSUM") as pwf,
            tc.tile_pool(name="pse", bufs=1, space="PSUM") as pe_,
        ):
            pc = cpool.tile([N, 260], bf16)
            nc.sync.dma_start(out=pc[:], in_=pconst_in[:])
            rc = cpool.tile([N, 195], f32)
            nc.sync.dma_start(out=rc[:], in_=rconst_in[:])
            sg = cpool.tile([N, 2 * N * CB], bf16)
            nc.sync.dma_start(out=sg[:], in_=sg_in[:])

            strips = cpool.tile([N, 12 * 160], bf16)
            nc.sync.dma_start(out=strips[:], in_=strips_in[:])
            qbasis = cpool.tile([3, N], f32)
            nc.sync.dma_start(out=qbasis[:], in_=qbasis_in[:])

            # clock warm-up: make ACT/DVE observe every setup DMA queue via
            # tiny reads, so later real instructions emit few sync waits
            # (walrus caps waits per instruction; vector clocks here are not
            # transitively collapsed).
            warm_s = cpool.tile([1, 4], f32, tag="warm_s")
            warm_v = cpool.tile([1, 4], f32, tag="warm_v")
            for idx, src in enumerate((strips, pc, rc, sg)):
                nc.scalar.copy(warm_s[0:1, idx:idx + 1], src[0:1, 0:1])
                nc.vector.tensor_copy(warm_v[0:1, idx:idx + 1], src[0:1, 0:1])

            vcs = cpool.tile([N, nwin], f32)
            ss = cpool.tile([N, nwin], f32)
            mus = cpool.tile([1, nwin], f32)

            for ch in range(nchunk):
                cw0 = ch * CB
                rq_sb = qpool.tile([3, CB * N], f32)
                nc.gpsimd.dma_start(
                    out=rq_sb[:], in_=rhsq_in[:, cw0 * N:(cw0 + CB) * N]
                )
                s_sb = qpool.tile([N, CB * N], f32, tag="s_sb")
                for h in range(3):
                    qps = pq_.tile([N, 390], f32)
                    nc.tensor.matmul(
                        qps[:], qbasis[:], rq_sb[:, h * 390:(h + 1) * 390],
                        start=True, stop=True,
                    )
                    # sqrt straight out of PSUM into the chunk-wide s tile
                    nc.scalar.activation(
                        s_sb[:, h * 390:(h + 1) * 390], qps[:], AF.Sqrt
                    )
                r_sb = rpool.tile([N, CB * N], f32)
                nc.scalar.activation(r_sb[:], s_sb[:], AF.Exp, scale=-1.0)

                wf_sb = wfpool.tile([N, CB * 130], bf16)
                e_sb = epool.tile([N, CB * N], bf16)

                for g in range(NGRP):
                    ps1p = pp1.tile([N, 390], f32)
                    ps1r = pr1.tile([N, 390], f32)
                    wfp = pwf.tile([N, 390], f32)
                    ep = pe_.tile([N, 195], f32)
                    for j in range(GRP):
                        w = cw0 + g * GRP + j
                        r0, c0 = divmod(w, 96)
                        lhs = strips[:, r0 * 160 + c0: r0 * 160 + c0 + N]
                        nc.tensor.matmul(
                            ps1p[:, j * 130:(j + 1) * 130], lhs, pc[:, 0:130],
                            start=True, stop=True,
                        )
                    o1p = o1ppool.tile([N, 390], bf16)
                    nc.scalar.copy(o1p[:], ps1p[:])
                    for j in range(GRP):
                        nc.tensor.matmul(
                            wfp[:, j * 130:(j + 1) * 130],
                            o1p[:, j * 130:j * 130 + N], pc[:, 0:130],
                            start=True, stop=False,
                        )
                        nc.tensor.matmul(
                            wfp[:, j * 130:(j + 1) * 130],
                            o1p[:, j * 130 + N:(j + 1) * 130], pc[:, 130:260],
                            start=False, stop=True,
                        )
                    nc.scalar.copy(wf_sb[:, g * 390:(g + 1) * 390], wfp[:])

                    for j in range(GRP):
                        wl = (g * GRP + j) * N
                        nc.tensor.matmul(
                            ps1r[:, j * 130:(j + 1) * 130],
                            r_sb[:, wl:wl + N], rc[:, 0:130],
                            start=True, stop=True,
                        )
                    o1r = o1rpool.tile([N, 390], f32)
                    nc.vector.tensor_copy(o1r[:], ps1r[:])
                    for j in range(GRP):
                        nc.tensor.matmul(
                            ep[:, j * N:(j + 1) * N],
                            o1r[:, j * 130:j * 130 + N], rc[:, 0:N],
                            start=True, stop=False,
                        )
                        nc.tensor.matmul(
                            ep[:, j * N:(j + 1) * N],
                            o1r[:, j * 130 + N:(j + 1) * 130], rc[:, 130:195],
                            start=False, stop=True,
                        )
                    nc.vector.tensor_copy(e_sb[:, g * 195:(g + 1) * 195], ep[:])

                # spectral chain, batched over the CB windows of this chunk
                FD = CB * N
                rp = spool.tile([N, FD], bf16, tag="rp")
                nc.vector.tensor_scalar(rp[:], e_sb[:], 1e-8, 0.0, op0=ALU.add, op1=ALU.max)
                mn = spool.tile([N, FD], bf16, tag="mn")
                nc.vector.tensor_scalar(mn[:], e_sb[:], 1e-8, 0.0, op0=ALU.add, op1=ALU.min)
                gp = spool.tile([N, FD], bf16, tag="gp")
                nc.scalar.activation(gp[:], rp[:], AF.Sqrt)
                gn = spool.tile([N, FD], bf16, tag="gn")
                nc.scalar.activation(gn[:], mn[:], AF.Sqrt, scale=-1.0)
                gps = spool.tile([N, FD], bf16, tag="gps")
                nc.vector.tensor_mul(gps[:], gp[:], sg[:, 0:FD])
                gns = spool.tile([N, FD], bf16, tag="gns")
                nc.vector.tensor_mul(gns[:], gn[:], sg[:, FD:2 * FD])

                wf3 = wf_sb[:].rearrange("p (w t c) -> p w t c", t=2, c=N)
                wr = wf3[:, :, 0, :]   # [65, CB, 65]
                wi = wf3[:, :, 1, :]
                gps3 = gps[:].rearrange("p (w c) -> p w c", c=N)
                gns3 = gns[:].rearrange("p (w c) -> p w c", c=N)
                a_t = spool.tile([N, FD], bf16, tag="a_t")
                a3 = a_t[:].rearrange("p (w c) -> p w c", c=N)
                nc.vector.tensor_mul(a3, wr, gps3)
                b_t = spool.tile([N, FD], bf16, tag="b_t")
                b3 = b_t[:].rearrange("p (w c) -> p w c", c=N)
                nc.vector.tensor_mul(b3, wi, gns3)
                xr = spool.tile([N, FD], bf16, tag="xr")
                nc.vector.tensor_sub(xr[:], a_t[:], b_t[:])

                wr2 = spool.tile([N, FD], bf16, tag="wr2")
                wr23 = wr2[:].rearrange("p (w c) -> p w c", c=N)
                nc.vector.tensor_mul(wr23, wr, wr)
                wi2 = spool.tile([N, FD], bf16, tag="wi2")
                wi23 = wi2[:].rearrange("p (w c) -> p w c", c=N)
                nc.vector.tensor_mul(wi23, wi, wi)
                w2s = spool.tile([N, FD], bf16, tag="w2s")
                nc.vector.tensor_add(w2s[:], wr2[:], wi2[:])
                az = spool.tile([N, FD], bf16, tag="az")
                nc.vector.tensor_sub(az[:], rp[:], mn[:])   # |E+eps| = relu(z) - min(z,0)
                st = spool.tile([N, FD], bf16, tag="st")
                nc.vector.tensor_mul(st[:], w2s[:], az[:])

                xr3 = xr[:].rearrange("p (w c) -> p w c", c=N)
                st3 = st[:].rearrange("p (w c) -> p w c", c=N)
                nc.vector.tensor_reduce(
                    vcs[:, cw0:cw0 + CB], xr3, axis=AX.X, op=ALU.add
                )
                nc.vector.tensor_reduce(
                    ss[:, cw0:cw0 + CB], st3, axis=AX.X, op=ALU.add
                )
                nc.vector.tensor_copy(mus[0:1, cw0:cw0 + CB], xr3[0:1, :, 0])

            nc.sync.dma_start(out=out_d[0:N, :], in_=vcs[:])
            nc.sync.dma_start(out=out_d[N:130, :], in_=ss[:])
            nc.sync.dma_start(out=out_d[130:131, :], in_=mus[:])
    if not nc.is_finalized():
        nc.finalize()
    return nc


def _host_inputs(angle_matrix, noise, nchunk):
    """Per-core input maps. Core c owns output rows [12c, 12c+12)."""
    pconst, rconst, sgconst = _host_constants()
    noise2d = np.ascontiguousarray(np.asarray(noise, dtype=np.float32)[0, 0])
    ang = np.asarray(angle_matrix, dtype=np.float32).reshape(-1).astype(np.float64)
    c = np.cos(ang); s = np.sin(ang)
    alpha = (c * c / A_**2 + s * s / B_**2).astype(np.float32)
    beta = (s * s / A_**2 + c * c / B_**2).astype(np.float32)
    gamma = (2 * c * s * (1 / A_**2 - 1 / B_**2)).astype(np.float32)
    x = np.linspace(-D, D, N, dtype=np.float32)
    x2 = x * x
    nwin = nchunk * CB
    qbasis = np.stack([x2, np.ones(N, np.float32), x]).astype(np.float32)  # [3, 65]

    in_maps = []
    for core in range(NCORE):
        w0 = core * WPC
        al = alpha[w0:w0 + nwin]
        be = beta[w0:w0 + nwin]
        ga = gamma[w0:w0 + nwin]
        # q[r, (w,c)] = x2[r]*rhsq[0] + 1*rhsq[1] + x[r]*rhsq[2]
        rhsq = np.empty((3, nwin, N), np.float32)
        rhsq[0] = al[:, None]
        rhsq[1] = be[:, None] * x2[None, :]
        rhsq[2] = ga[:, None] * x[None, :]
        r_base = core * RPC
        strips = np.concatenate(
            [noise2d[r_base + i: r_base + i + N, :] for i in range(12)], axis=1
        ).astype(_bf16)
        in_maps.append({
            "strips": strips,
            "rhsq": rhsq.reshape(3, -1),
            "qbasis": qbasis,
            "pconst": pconst,
            "rconst": rconst,
            "sgconst": sgconst,
        })
    return in_maps


def _finalize(core_outs):
    """core_outs: list of [131, nwin] arrays -> [96, 96] output."""
    blocks = []
    for arr in core_outs:
        vc = arr[0:N].sum(axis=0)
        s_ = arr[N:130].sum(axis=0)
        mu = arr[130]
        vcn = (vc - mu) / N2
        var = (s_ - mu * mu) / (N2 * (N2 - 1.0))
        outrow = vcn / (np.sqrt(np.maximum(var, 0.0)) + 1e-6)
        blocks.append(outrow.reshape(-1, 96))
    return np.concatenate(blocks, axis=0).astype(np.float32)


_PROG = {}
_RUNNER = {}


def _get_program(nchunk):
    if nchunk not in _PROG:
        _PROG[nchunk] = _build_program(nchunk)
    return _PROG[nchunk]


def _get_runner(nchunk):
    """Build (once) a jitted shard_map executable over the 8 cores.

    Mirrors concourse.bass2jax.run_bass_via_pjrt, but caches the traced/
    compiled callable so repeat kernel() calls skip retracing.
    """
    if nchunk in _RUNNER:
        return _RUNNER[nchunk]
    import jax
    import concourse.mybir as mybir
    from concourse import bass2jax
    from jax.experimental.shard_map import shard_map
    from jax.sharding import Mesh, PartitionSpec

    nc = _get_program(nchunk)
    bass2jax.install_neuronx_cc_hook()
    assert nc.dbg_addr is None
    partition_name = (
        nc.partition_id_tensor.name if nc.partition_id_tensor else None
    )
    in_names, out_names, out_avals, zero_outs = [], [], [], []
    for alloc in nc.m.functions[0].allocations:
        if not isinstance(alloc, mybir.MemoryLocationSet):
            continue
        name = alloc.memorylocations[0].name
        if alloc.kind == "ExternalInput":
            if name != partition_name:
                in_names.append(name)
        elif alloc.kind == "ExternalOutput":
            out_names.append(name)
            shape = tuple(alloc.tensor_shape)
            dtype = mybir.dt.np(alloc.dtype)
            out_avals.append(jax.core.ShapedArray(shape, dtype))
            zero_outs.append(np.zeros(shape, dtype))
    n_params = len(in_names)
    n_outs = len(out_avals)
    in_names_all = in_names + out_names
    if partition_name is not None:
        in_names_all.append(partition_name)
    donate = tuple(range(n_params, n_params + n_outs))

    def _body(*args):
        operands = list(args)
        if partition_name is not None:
            operands.append(bass2jax.partition_id_tensor())
        outs = bass2jax._bass_exec_p.bind(
            *operands,
            out_avals=tuple(out_avals),
            in_names=tuple(in_names_all),
            out_names=tuple(out_names),
            lowering_input_output_aliases=(),
            sim_require_finite=True,
            sim_require_nnan=True,
            nc=nc,
        )
        return tuple(outs)

    devices = jax.devices()[:NCORE]
    mesh = Mesh(np.asarray(devices), ("core",))
    sharded = jax.jit(
        shard_map(
            _body,
            mesh=mesh,
            in_specs=(PartitionSpec("core"),) * (n_params + n_outs),
            out_specs=(PartitionSpec("core"),) * n_outs,
            check_rep=False,
        ),
        donate_argnums=donate,
        keep_unused=True,
    )
    zero_concats = [
        np.zeros((NCORE * z.shape[0], *z.shape[1:]), z.dtype) for z in zero_outs
    ]
    info = (sharded, in_names, out_names, out_avals, zero_concats)
    _RUNNER[nchunk] = info
    return info


def _run(in_maps, nchunk):
    sharded, in_names, out_names, out_avals, zero_concats = _get_runner(nchunk)
    concat_in = [
        np.concatenate([in_maps[c][name] for c in range(NCORE)], axis=0)
        for name in in_names
    ]
    out_arrs = sharded(*concat_in, *[z.copy() for z in zero_concats])
    outs = {
        name: np.asarray(out_arrs[i]).reshape(NCORE, *out_avals[i].shape)
        for i, name in enumerate(out_names)
    }
    return [outs["out"][c] for c in range(NCORE)]


def kernel(angle_matrix, noise):
    nchunk = WPC // CB
    in_maps = _host_inputs(angle_matrix, noise, nchunk)
    core_outs = _run(in_maps, nchunk)
    return _finalize(core_outs)



# revision 8
# speedup vs baseline: 8.9062x; 8.9062x over previous
"""Trainium2 Bass kernel for nn_BatchFFTMA: 9216 independent 65x65 FFT-MA sims.

Math (validated against the jax reference):
  For each window w (patch p = noise[r0:r0+65, c0:c0+65], angle theta):
    Wf' = Cpt^T p Cpt   with Cpt = F*diag((-1)^k)  (patch DFT; center-pixel
          phase e^{2pi i 32k/65} and the principal-sqrt half-shift phase
          combine to exactly (-1)^k)
    E   = Re(Cq^T R Cq) with Cq = F*diag(e^{-2pi i 33 k/65})  (DFT of the
          ifftshift-aligned R -> real, even spectrum; the reference's
          fftshift-vs-ifftshift off-by-one is the source of the half-shift)
    R   = exp(-sqrt(q)), q = alpha*x_r^2 + beta*x_c^2 + gamma*x_r*x_c
    gp  = sqrt(relu(E+1e-8)), gn = sqrt(relu(-(E+1e-8)))
    g+  = gp*SGP, g- = gn*SGN   (SGP/SGN: +-1 fields from sqrt branch cuts)
    a = Wf'_r*g+, b = Wf'_i*g-; Xr = a - b
    VC = sum(Xr); X00 = Xr[0,0]; S = sum((Wf'_r^2+Wf'_i^2)*|E+1e-8|)
    out_w = ((VC-X00)/N^2) / (sqrt((S-X00^2)/(N^2(N^2-1))) + 1e-6)
  (v = ifft2 never materialized: center pixel via phase fold, mean via X[0,0],
   std via Parseval.)

Sharding: window/batch axis across 8 cores (1152 windows each, 12 output rows).
Precision: patch DFT + spectral chain bf16 (validated), R field + R DFT fp32.

I/O strategy (axon tunnel: ~82ms fixed RTT + ~11ms/MB, so payload rules):
  - constants (DFT matrices, sign fields, bases) committed to device once
    at runner build; zero per-call transfer.
  - per-call payload per core: noise rows [76,160] bf16 (24KB) + per-window
    alpha/beta/gamma [3,1152] f32 (14KB). Strips, the covariance rhs, and
    everything else are built on device.
  - final normalize runs on device; output is [1,1152] f32 per core (4.6KB).
"""
import numpy as np
import ml_dtypes

H, W, D = 96, 96, 32
N = 65
N2 = N * N
A_, B_ = 15.0, 3.0
NCORE = 8
WPC = H * W // NCORE      # 1152 windows per core
RPC = H // NCORE          # 12 output rows per core
CB = 18                   # windows per vector chunk
GRP = 3                   # windows per matmul/PSUM group
NGRP = CB // GRP

_bf16 = ml_dtypes.bfloat16


def _host_constants():
    k = np.arange(N)
    F = np.exp(-2j * np.pi * np.outer(k, k) / N)
    Cpt = F * ((-1.0) ** k)[None, :]
    Cq = F * np.exp(-2j * np.pi * k * 33 / N)[None, :]
    Cr = Cpt.real.astype(np.float32)
    Ci = Cpt.imag.astype(np.float32)
    Qr = Cq.real.astype(np.float32)
    Qi = Cq.imag.astype(np.float32)
    pconst = np.concatenate([Cr, Ci, -Ci, Cr], axis=1).astype(_bf16)   # [65, 260]
    rconst = np.concatenate([Qr, Qi, -Qi], axis=1).astype(np.float32)  # [65, 195]

    k1, k2 = np.meshgrid(k, k, indexing="ij")
    ksum = k1 + k2
    m = ksum % N
    extra = (-1.0) ** (ksum // N)
    SGP = extra * np.where(m <= 32, 1.0, -1.0)
    SGN = extra * np.where(m == 0, 1.0, -1.0)
    sgp_t = np.tile(SGP, (1, CB)).astype(_bf16)   # [65, 65*CB]
    sgn_t = np.tile(SGN, (1, CB)).astype(_bf16)
    sgconst = np.concatenate([sgp_t, sgn_t], axis=1)  # [65, 2*65*CB]

    x = np.linspace(-D, D, N, dtype=np.float32)
    x2 = x * x
    ones = np.ones(N, np.float32)
    # q[r, (w,c)] = x2[r]*rq[0] + 1*rq[1] + x[r]*rq[2]
    # rq[h, (w,c)] = abg[h, w] * basis2[h, c]
    qbasis = np.stack([x2, ones, x]).astype(np.float32)   # [3, 65]
    basis2 = np.stack([ones, x2, x]).astype(np.float32)   # [3, 65]
    ones65 = np.ones((N, 1), np.float32)
    return pconst, rconst, sgconst, qbasis, basis2, ones65


def _build_program(nchunk):
    import concourse.bacc as bacc
    import concourse.mybir as mybir
    from concourse.tile import TileContext

    f32 = mybir.dt.float32
    bf16 = mybir.dt.bfloat16
    AF = mybir.ActivationFunctionType
    ALU = mybir.AluOpType
    AX = mybir.AxisListType

    nwin = nchunk * CB
    nc = bacc.Bacc()
    nz_in = nc.declare_dram_parameter("nz", [RPC + N - 1, 160], bf16, isOutput=False)
    abg_in = nc.declare_dram_parameter("abg", [3, nwin], f32, isOutput=False)
    pconst_in = nc.declare_dram_parameter("pconst", [N, 260], bf16, isOutput=False)
    rconst_in = nc.declare_dram_parameter("rconst", [N, 195], f32, isOutput=False)
    sg_in = nc.declare_dram_parameter("sgconst", [N, 2 * N * CB], bf16, isOutput=False)
    qbasis_in = nc.declare_dram_parameter("qbasis", [3, N], f32, isOutput=False)
    basis2_in = nc.declare_dram_parameter("basis2", [3, N], f32, isOutput=False)
    ones_in = nc.declare_dram_parameter("ones65", [N, 1], f32, isOutput=False)
    out_d = nc.declare_dram_parameter("out", [1, nwin], f32, isOutput=True)

    with TileContext(nc) as tc:
        with (
            tc.tile_pool(name="const", bufs=1) as cpool,
            tc.tile_pool(name="qsb", bufs=3) as qpool,
            tc.tile_pool(name="rsb", bufs=2) as rpool,
            tc.tile_pool(name="o1p", bufs=2) as o1ppool,
            tc.tile_pool(name="o1r", bufs=2) as o1rpool,
            tc.tile_pool(name="wfsb", bufs=2) as wfpool,
            tc.tile_pool(name="esb", bufs=2) as epool,
            tc.tile_pool(name="spec", bufs=2) as spool,
            tc.tile_pool(name="psq", bufs=2, space="PSUM") as pq_,
            tc.tile_pool(name="ps1p", bufs=2, space="PSUM") as pp1,
            tc.tile_pool(name="ps1r", bufs=2, space="PSUM") as pr1,
            tc.tile_pool(name="pswf", bufs=1, space="PSUM") as pwf,
            tc.tile_pool(name="pse", bufs=1, space="PSUM") as pe_,
        ):
            pc = cpool.tile([N, 260], bf16)
            nc.sync.dma_start(out=pc[:], in_=pconst_in[:])
            rc = cpool.tile([N, 195], f32)
            nc.sync.dma_start(out=rc[:], in_=rconst_in[:])
            sg = cpool.tile([N, 2 * N * CB], bf16)
            nc.sync.dma_start(out=sg[:], in_=sg_in[:])
            qbasis = cpool.tile([3, N], f32)
            nc.sync.dma_start(out=qbasis[:], in_=qbasis_in[:])
            basis2 = cpool.tile([3, N], f32)
            nc.sync.dma_start(out=basis2[:], in_=basis2_in[:])
            ones65 = cpool.tile([N, 1], f32)
            nc.sync.dma_start(out=ones65[:], in_=ones_in[:])
            abg = cpool.tile([3, nwin], f32)
            nc.sync.dma_start(out=abg[:], in_=abg_in[:])

            # sliding-window row strips, built on device from the compact
            # [76, 160] noise block: strip i = noise rows [i, i+65)
            strips = cpool.tile([N, RPC * 160], bf16)
            for i in range(RPC):
                nc.sync.dma_start(
                    out=strips[:, i * 160:(i + 1) * 160], in_=nz_in[i:i + N, :]
                )

            # clock warm-up: make ACT/DVE observe every setup DMA queue via
            # tiny reads, so later real instructions emit few sync waits
            # (walrus caps waits per instruction; vector clocks here are not
            # transitively collapsed).
            warm_s = cpool.tile([1, 8], f32, tag="warm_s")
            warm_v = cpool.tile([1, 8], f32, tag="warm_v")
            for idx, src in enumerate(
                (strips, pc, rc, sg, qbasis, basis2, ones65, abg)
            ):
                nc.scalar.copy(warm_s[0:1, idx:idx + 1], src[0:1, 0:1])
                nc.vector.tensor_copy(warm_v[0:1, idx:idx + 1], src[0:1, 0:1])

            vcs = cpool.tile([N, nwin], f32)
            ss = cpool.tile([N, nwin], f32)
            mus = cpool.tile([1, nwin], f32)

            for ch in range(nchunk):
                cw0 = ch * CB
                # rq[h, (w,c)] = abg[h, w] * basis2[h, c]  (broadcast APs)
                rq_sb = qpool.tile([3, CB * N], f32)
                rq3 = rq_sb[:].rearrange("p (w c) -> p w c", c=N)
                abg_v = abg[:, cw0:cw0 + CB].unsqueeze(2).broadcast_to([3, CB, N])
                b2_v = basis2[:].unsqueeze(1).broadcast_to([3, CB, N])
                nc.vector.tensor_tensor(rq3, abg_v, b2_v, op=ALU.mult)

                s_sb = qpool.tile([N, CB * N], f32, tag="s_sb")
                for h in range(3):
                    qps = pq_.tile([N, 390], f32)
                    nc.tensor.matmul(
                        qps[:], qbasis[:], rq_sb[:, h * 390:(h + 1) * 390],
                        start=True, stop=True,
                    )
                    # sqrt straight out of PSUM into the chunk-wide s tile
                    nc.scalar.activation(
                        s_sb[:, h * 390:(h + 1) * 390], qps[:], AF.Sqrt
                    )
                r_sb = rpool.tile([N, CB * N], f32)
                nc.scalar.activation(r_sb[:], s_sb[:], AF.Exp, scale=-1.0)

                wf_sb = wfpool.tile([N, CB * 130], bf16)
                e_sb = epool.tile([N, CB * N], bf16)

                for g in range(NGRP):
                    ps1p = pp1.tile([N, 390], f32)
                    ps1r = pr1.tile([N, 390], f32)
                    wfp = pwf.tile([N, 390], f32)
                    ep = pe_.tile([N, 195], f32)
                    for j in range(GRP):
                        w = cw0 + g * GRP + j
                        r0, c0 = divmod(w, 96)
                        lhs = strips[:, r0 * 160 + c0: r0 * 160 + c0 + N]
                        nc.tensor.matmul(
                            ps1p[:, j * 130:(j + 1) * 130], lhs, pc[:, 0:130],
                            start=True, stop=True,
                        )
                    o1p = o1ppool.tile([N, 390], bf16)
                    nc.scalar.copy(o1p[:], ps1p[:])
                    for j in range(GRP):
                        nc.tensor.matmul(
                            wfp[:, j * 130:(j + 1) * 130],
                            o1p[:, j * 130:j * 130 + N], pc[:, 0:130],
                            start=True, stop=False,
                        )
                        nc.tensor.matmul(
                            wfp[:, j * 130:(j + 1) * 130],
                            o1p[:, j * 130 + N:(j + 1) * 130], pc[:, 130:260],
                            start=False, stop=True,
                        )
                    nc.scalar.copy(wf_sb[:, g * 390:(g + 1) * 390], wfp[:])

                    for j in range(GRP):
                        wl = (g * GRP + j) * N
                        nc.tensor.matmul(
                            ps1r[:, j * 130:(j + 1) * 130],
                            r_sb[:, wl:wl + N], rc[:, 0:130],
                            start=True, stop=True,
                        )
                    o1r = o1rpool.tile([N, 390], f32)
                    nc.vector.tensor_copy(o1r[:], ps1r[:])
                    for j in range(GRP):
                        nc.tensor.matmul(
                            ep[:, j * N:(j + 1) * N],
                            o1r[:, j * 130:j * 130 + N], rc[:, 0:N],
                            start=True, stop=False,
                        )
                        nc.tensor.matmul(
                            ep[:, j * N:(j + 1) * N],
                            o1r[:, j * 130 + N:(j + 1) * 130], rc[:, 130:195],
                            start=False, stop=True,
                        )
                    nc.vector.tensor_copy(e_sb[:, g * 195:(g + 1) * 195], ep[:])

                # spectral chain, batched over the CB windows of this chunk
                FD = CB * N
                rp = spool.tile([N, FD], bf16, tag="rp")
                nc.vector.tensor_scalar(rp[:], e_sb[:], 1e-8, 0.0, op0=ALU.add, op1=ALU.max)
                mn = spool.tile([N, FD], bf16, tag="mn")
                nc.vector.tensor_scalar(mn[:], e_sb[:], 1e-8, 0.0, op0=ALU.add, op1=ALU.min)
                gp = spool.tile([N, FD], bf16, tag="gp")
                nc.scalar.activation(gp[:], rp[:], AF.Sqrt)
                gn = spool.tile([N, FD], bf16, tag="gn")
                nc.scalar.activation(gn[:], mn[:], AF.Sqrt, scale=-1.0)
                gps = spool.tile([N, FD], bf16, tag="gps")
                nc.vector.tensor_mul(gps[:], gp[:], sg[:, 0:FD])
                gns = spool.tile([N, FD], bf16, tag="gns")
                nc.vector.tensor_mul(gns[:], gn[:], sg[:, FD:2 * FD])

                wf3 = wf_sb[:].rearrange("p (w t c) -> p w t c", t=2, c=N)
                wr = wf3[:, :, 0, :]   # [65, CB, 65]
                wi = wf3[:, :, 1, :]
                gps3 = gps[:].rearrange("p (w c) -> p w c", c=N)
                gns3 = gns[:].rearrange("p (w c) -> p w c", c=N)
                a_t = spool.tile([N, FD], bf16, tag="a_t")
                a3 = a_t[:].rearrange("p (w c) -> p w c", c=N)
                nc.vector.tensor_mul(a3, wr, gps3)
                b_t = spool.tile([N, FD], bf16, tag="b_t")
                b3 = b_t[:].rearrange("p (w c) -> p w c", c=N)
                nc.vector.tensor_mul(b3, wi, gns3)
                xr = spool.tile([N, FD], bf16, tag="xr")
                nc.vector.tensor_sub(xr[:], a_t[:], b_t[:])

                wr2 = spool.tile([N, FD], bf16, tag="wr2")
                wr23 = wr2[:].rearrange("p (w c) -> p w c", c=N)
                nc.vector.tensor_mul(wr23, wr, wr)
                wi2 = spool.tile([N, FD], bf16, tag="wi2")
                wi23 = wi2[:].rearrange("p (w c) -> p w c", c=N)
                nc.vector.tensor_mul(wi23, wi, wi)
                w2s = spool.tile([N, FD], bf16, tag="w2s")
                nc.vector.tensor_add(w2s[:], wr2[:], wi2[:])
                az = spool.tile([N, FD], bf16, tag="az")
                nc.vector.tensor_sub(az[:], rp[:], mn[:])   # |E+eps| = relu(z) - min(z,0)
                st = spool.tile([N, FD], bf16, tag="st")
                nc.vector.tensor_mul(st[:], w2s[:], az[:])

                xr3 = xr[:].rearrange("p (w c) -> p w c", c=N)
                st3 = st[:].rearrange("p (w c) -> p w c", c=N)
                nc.vector.tensor_reduce(
                    vcs[:, cw0:cw0 + CB], xr3, axis=AX.X, op=ALU.add
                )
                nc.vector.tensor_reduce(
                    ss[:, cw0:cw0 + CB], st3, axis=AX.X, op=ALU.add
                )
                nc.vector.tensor_copy(mus[0:1, cw0:cw0 + CB], xr3[0:1, :, 0])

            # ---- on-device finalize: one scalar per window ----
            # VC = colsum(vcs), S = colsum(ss) via ones-matmul (contract over
            # the 65 partition rows); 384-col blocks fit one PSUM bank.
            vc_row = cpool.tile([1, nwin], f32, tag="vc_row")
            s_row = cpool.tile([1, nwin], f32, tag="s_row")
            NB = 384
            for b0 in range(0, nwin, NB):
                bn = min(NB, nwin - b0)
                pv = pq_.tile([1, NB], f32, tag="qps")
                nc.tensor.matmul(
                    pv[:, 0:bn], ones65[:], vcs[:, b0:b0 + bn],
                    start=True, stop=True,
                )
                nc.vector.tensor_copy(vc_row[0:1, b0:b0 + bn], pv[:, 0:bn])
                pv2 = pq_.tile([1, NB], f32, tag="qps")
                nc.tensor.matmul(
                    pv2[:, 0:bn], ones65[:], ss[:, b0:b0 + bn],
                    start=True, stop=True,
                )
                nc.vector.tensor_copy(s_row[0:1, b0:b0 + bn], pv2[:, 0:bn])

            # out = ((VC-mu)/N2) / (sqrt(max(S-mu^2,0)/(N2*(N2-1))) + 1e-6)
            num = cpool.tile([1, nwin], f32, tag="num")
            nc.vector.tensor_sub(num[:], vc_row[:], mus[:])
            m2 = cpool.tile([1, nwin], f32, tag="m2")
            nc.vector.tensor_mul(m2[:], mus[:], mus[:])
            varn = cpool.tile([1, nwin], f32, tag="varn")
            nc.vector.tensor_sub(varn[:], s_row[:], m2[:])
            nc.vector.tensor_scalar_max(varn[:], varn[:], 0.0)
            den = cpool.tile([1, nwin], f32, tag="den")
            c2 = 1.0 / (float(N2) * (N2 - 1.0))
            nc.scalar.activation(den[:], varn[:], AF.Sqrt, scale=c2)
            nc.vector.tensor_scalar_add(den[:], den[:], 1e-6)
            rec = cpool.tile([1, nwin], f32, tag="rec")
            nc.vector.reciprocal(rec[:], den[:])
            outrow = cpool.tile([1, nwin], f32, tag="outrow")
            nc.vector.scalar_tensor_tensor(
                outrow[:], num[:], 1.0 / N2, rec[:], op0=ALU.mult, op1=ALU.mult
            )
            nc.sync.dma_start(out=out_d[:], in_=outrow[:])
    if not nc.is_finalized():
        nc.finalize()
    return nc


_HOST_CONSTS = None


def _host_inputs(angle_matrix, noise, nchunk):
    """Per-core input maps. Core c owns output rows [12c, 12c+12)."""
    global _HOST_CONSTS
    if _HOST_CONSTS is None:
        _HOST_CONSTS = _host_constants()
    pconst, rconst, sgconst, qbasis, basis2, ones65 = _HOST_CONSTS
    noise2d = np.asarray(noise, dtype=np.float32)[0, 0]
    nzb = noise2d.astype(_bf16)
    ang = np.asarray(angle_matrix, dtype=np.float32).reshape(-1).astype(np.float64)
    c = np.cos(ang); s = np.sin(ang)
    alpha = (c * c / A_**2 + s * s / B_**2).astype(np.float32)
    beta = (s * s / A_**2 + c * c / B_**2).astype(np.float32)
    gamma = (2 * c * s * (1 / A_**2 - 1 / B_**2)).astype(np.float32)
    abg_all = np.stack([alpha, beta, gamma])  # [3, 9216]

    in_maps = []
    for core in range(NCORE):
        w0 = core * WPC
        r_base = core * RPC
        in_maps.append({
            "nz": np.ascontiguousarray(nzb[r_base:r_base + RPC + N - 1, :]),
            "abg": np.ascontiguousarray(abg_all[:, w0:w0 + WPC]),
            "pconst": pconst,
            "rconst": rconst,
            "sgconst": sgconst,
            "qbasis": qbasis,
            "basis2": basis2,
            "ones65": ones65,
        })
    return in_maps


def _finalize(core_outs):
    """core_outs: list of [1, 1152] arrays -> [96, 96] output."""
    return np.concatenate(
        [arr.reshape(RPC, 96) for arr in core_outs], axis=0
    ).astype(np.float32)


_PROG = {}
_RUNNER = {}
_CONST_NAMES = frozenset(
    {"pconst", "rconst", "sgconst", "qbasis", "basis2", "ones65"}
)
_DEV_CONSTS = {}


def _get_program(nchunk):
    if nchunk not in _PROG:
        _PROG[nchunk] = _build_program(nchunk)
    return _PROG[nchunk]


def _get_runner(nchunk):
    """Build (once) a jitted shard_map executable over the 8 cores.

    Mirrors concourse.bass2jax.run_bass_via_pjrt, but caches the traced/
    compiled callable so repeat kernel() calls skip retracing.
    """
    if nchunk in _RUNNER:
        return _RUNNER[nchunk]
    import jax
    import concourse.mybir as mybir
    from concourse import bass2jax
    from jax.experimental.shard_map import shard_map
    from jax.sharding import Mesh, PartitionSpec

    nc = _get_program(nchunk)
    bass2jax.install_neuronx_cc_hook()
    assert nc.dbg_addr is None
    partition_name = (
        nc.partition_id_tensor.name if nc.partition_id_tensor else None
    )
    in_names, out_names, out_avals, zero_outs = [], [], [], []
    for alloc in nc.m.functions[0].allocations:
        if not isinstance(alloc, mybir.MemoryLocationSet):
            continue
        name = alloc.memorylocations[0].name
        if alloc.kind == "ExternalInput":
            if name != partition_name:
                in_names.append(name)
        elif alloc.kind == "ExternalOutput":
            out_names.append(name)
            shape = tuple(alloc.tensor_shape)
            dtype = mybir.dt.np(alloc.dtype)
            out_avals.append(jax.core.ShapedArray(shape, dtype))
            zero_outs.append(np.zeros(shape, dtype))
    n_params = len(in_names)
    n_outs = len(out_avals)
    in_names_all = in_names + out_names
    if partition_name is not None:
        in_names_all.append(partition_name)
    donate = tuple(range(n_params, n_params + n_outs))

    def _body(*args):
        operands = list(args)
        if partition_name is not None:
            operands.append(bass2jax.partition_id_tensor())
        outs = bass2jax._bass_exec_p.bind(
            *operands,
            out_avals=tuple(out_avals),
            in_names=tuple(in_names_all),
            out_names=tuple(out_names),
            lowering_input_output_aliases=(),
            sim_require_finite=True,
            sim_require_nnan=True,
            nc=nc,
        )
        return tuple(outs)

    devices = jax.devices()[:NCORE]
    mesh = Mesh(np.asarray(devices), ("core",))
    sharded = jax.jit(
        shard_map(
            _body,
            mesh=mesh,
            in_specs=(PartitionSpec("core"),) * (n_params + n_outs),
            out_specs=(PartitionSpec("core"),) * n_outs,
            check_rep=False,
        ),
        donate_argnums=donate,
        keep_unused=True,
    )
    zero_concats = [
        np.zeros((NCORE * z.shape[0], *z.shape[1:]), z.dtype) for z in zero_outs
    ]
    info = (sharded, in_names, out_names, out_avals, zero_concats, mesh)
    _RUNNER[nchunk] = info
    return info


def _run(in_maps, nchunk):
    import jax
    from jax.sharding import NamedSharding, PartitionSpec

    sharded, in_names, out_names, out_avals, zero_concats, mesh = _get_runner(
        nchunk
    )
    args = []
    for name in in_names:
        if name in _CONST_NAMES:
            # constants live on device across calls: zero per-call transfer
            if name not in _DEV_CONSTS:
                concat = np.concatenate(
                    [in_maps[c][name] for c in range(NCORE)], axis=0
                )
                arr = jax.device_put(
                    concat, NamedSharding(mesh, PartitionSpec("core"))
                )
                arr.block_until_ready()
                _DEV_CONSTS[name] = arr
            args.append(_DEV_CONSTS[name])
        else:
            args.append(
                np.concatenate([in_maps[c][name] for c in range(NCORE)], axis=0)
            )
    out_arrs = sharded(*args, *[z.copy() for z in zero_concats])
    outs = {
        name: np.asarray(out_arrs[i]).reshape(NCORE, *out_avals[i].shape)
        for i, name in enumerate(out_names)
    }
    return [outs["out"][c] for c in range(NCORE)]


def kernel(angle_matrix, noise):
    nchunk = WPC // CB
    in_maps = _host_inputs(angle_matrix, noise, nchunk)
    core_outs = _run(in_maps, nchunk)
    return _finalize(core_outs)


# revision 17
# speedup vs baseline: 9.3930x; 1.0547x over previous
"""Trainium2 Bass kernel for nn_BatchFFTMA: 9216 independent 65x65 FFT-MA sims.

Math (validated against the jax reference; see v1 docstring for the
derivation of the phase folds):
  For each window w (patch p = noise[r0:r0+65, c0:c0+65], angle theta):
    Wf' = Cpt^T p Cpt   with Cpt = F*diag((-1)^k)
    E   = Re(Cq^T R Cq) with Cq = F*diag(e^{-2pi i 33 k/65})
    R   = exp(-sqrt(q)), q = alpha*x_r^2 + beta*x_c^2 + gamma*x_r*x_c
    gp  = sqrt(relu(E+1e-8)), gn = sqrt(relu(-(E+1e-8)))
    Xr  = Wf'_r*gp*SGP - Wf'_i*gn*SGN   (SGP/SGN: +-1 branch-cut fields)
    VC = sum(Xr); X00 = Xr[0,0]; S = sum((Wf'_r^2+Wf'_i^2)*|E+1e-8|)
    out_w = ((VC-X00)/N^2) / (sqrt((S-X00^2)/(N^2(N^2-1))) + 1e-6)

v2 compute structure (per core, 1152 windows, CB=18 windows/chunk):
  stage1 per window (lhsT = data):  o1p = P^T [Cr33|Ci33] (bf16),
                                    o1r = R^T [Qr33|Qi33] (f32)
  stage2 batched (lhsT = consts):   wfT_re/im = Cr^T o1pA -/+ ..., (bf16)
                                    E^T = Qr^T o1rA - Qi^T o1rB   (f32r)
  -> spectra come out transposed; all downstream sums are transpose-
  invariant. The spectral chain runs on halved columns k1=0..32 with x2
  weights folded into the SGP/SGN constants: the fields are Hermitian-even
  under joint index negation except on the k1+k2=65 line, where the E<0
  branch makes Xr odd (net zero) -> SGN weights are zeroed there.
  sqrt/exp phases batch over SC=4 chunks so the ACT table set switches only
  twice per superchunk (no table fits both sqrt and exp).

I/O strategy (axon tunnel: ~50-80ms fixed RTT + ~11ms/MB, so payload rules):
  constants committed to device once at runner build; per-call payload per
  core = noise rows [76,160] bf16 (24KB) + alpha/beta/gamma [3,1152] f32
  (14KB); final normalize on device, output [1,1152] f32 per core.
"""
import numpy as np
import ml_dtypes

H, W, D = 96, 96, 32
N = 65
N2 = N * N
A_, B_ = 15.0, 3.0
NCORE = 8
WPC = H * W // NCORE      # 1152 windows per core
RPC = H // NCORE          # 12 output rows per core
CB = 18                   # windows per chunk
GRP = 6                   # windows per stage1 matmul/PSUM group
NGRP = CB // GRP
BW = 9                    # windows per stage2 block
NH = 34                   # halved spectral columns (even, incl. both of the 32/33 mirror pair)
SC = 4                    # chunks per superchunk (ACT table batching)

_bf16 = ml_dtypes.bfloat16


def _host_constants():
    k = np.arange(N)
    F = np.exp(-2j * np.pi * np.outer(k, k) / N)
    Cpt = F * ((-1.0) ** k)[None, :]
    Cq = F * np.exp(-2j * np.pi * k * 33 / N)[None, :]
    Cr = Cpt.real.astype(np.float32)
    Ci = Cpt.imag.astype(np.float32)
    Qr = Cq.real.astype(np.float32)
    Qi = Cq.imag.astype(np.float32)
    pconst = np.concatenate(
        [Cr[:, :NH], Ci[:, :NH], Cr, Ci, -Ci], axis=1
    ).astype(_bf16)                                                # [65, 261]
    rconst = np.concatenate(
        [Qr[:, :NH], Qi[:, :NH], Qr, -Qi], axis=1
    ).astype(np.float32)                                           # [65, 196]

    k1, k2 = np.meshgrid(k, k, indexing="ij")
    ksum = k1 + k2
    m = ksum % N
    extra = (-1.0) ** (ksum // N)
    SGP = extra * np.where(m <= 32, 1.0, -1.0)
    SGN = extra * np.where(m == 0, 1.0, -1.0)
    wgt = np.ones(NH); wgt[1:32] = 2.0   # n=0,32,33 -> 1; n=1..31 -> 2
    SGP_h = SGP[:, :NH] * wgt[None, :]
    SGN_h = SGN[:, :NH] * wgt[None, :]
    for n in range(1, 32):
        SGN_h[(N - n) % N, n] = 0.0   # k1+k2=65 line: odd, net zero (mirror absent)
    sgp_t = np.tile(SGP_h, (1, CB)).astype(_bf16)   # [65, NH*CB]
    sgn_t = np.tile(SGN_h, (1, CB)).astype(_bf16)
    sgconst = np.concatenate([sgp_t, sgn_t], axis=1)  # [65, 2*NH*CB]
    wgt33 = np.tile(wgt[None, :], (N, 1)).astype(_bf16)  # [65, 33]

    x = np.linspace(-D, D, N, dtype=np.float32)
    x2 = x * x
    ones = np.ones(N, np.float32)
    # q[r, (w,c)] = x2[r]*rq[0] + 1*rq[1] + x[r]*rq[2]
    # rq[h, (w,c)] = abg[h, w] * basis2[h, c]
    qbasis = np.stack([x2, ones, x]).astype(np.float32)   # [3, 65]
    basis2 = np.stack([ones, x2, x]).astype(np.float32)   # [3, 65]
    ones65 = np.ones((N, 1), np.float32)
    return pconst, rconst, sgconst, wgt33, qbasis, basis2, ones65


def _build_program(nchunk):
    import concourse.bacc as bacc
    import concourse.mybir as mybir
    from concourse.tile import TileContext

    f32 = mybir.dt.float32
    f32r = mybir.dt.float32r
    bf16 = mybir.dt.bfloat16
    AF = mybir.ActivationFunctionType
    ALU = mybir.AluOpType
    AX = mybir.AxisListType

    nwin = nchunk * CB
    assert nchunk % SC == 0

    nc = bacc.Bacc()
    nz_in = nc.declare_dram_parameter("nz", [RPC + N - 1, 160], bf16, isOutput=False)
    abg_in = nc.declare_dram_parameter("abg", [3, nwin], f32, isOutput=False)
    W2 = 2 * NH
    pconst_in = nc.declare_dram_parameter("pconst", [N, W2 + 195], bf16, isOutput=False)
    rconst_in = nc.declare_dram_parameter("rconst", [N, W2 + 130], f32, isOutput=False)
    sg_in = nc.declare_dram_parameter("sgconst", [N, 2 * NH * CB], bf16, isOutput=False)
    wgt_in = nc.declare_dram_parameter("wgt33", [N, NH], bf16, isOutput=False)
    qbasis_in = nc.declare_dram_parameter("qbasis", [3, N], f32, isOutput=False)
    basis2_in = nc.declare_dram_parameter("basis2", [3, N], f32, isOutput=False)
    ones_in = nc.declare_dram_parameter("ones65", [N, 1], f32, isOutput=False)
    out_d = nc.declare_dram_parameter("out", [1, nwin], f32, isOutput=True)

    with TileContext(nc) as tc:
        with (
            tc.tile_pool(name="const", bufs=1) as cpool,
            tc.tile_pool(name="qsb", bufs=2) as qpool,
            tc.tile_pool(name="rqsb", bufs=SC + 1) as rqpool,
            tc.tile_pool(name="rsb", bufs=2) as rpool,
            tc.tile_pool(name="o1sb", bufs=2) as o1pool,
            tc.tile_pool(name="wfsb", bufs=2) as wfpool,
            tc.tile_pool(name="spec", bufs=2) as spool,
            tc.tile_pool(name="psq", bufs=2, space="PSUM") as pq_,
            tc.tile_pool(name="ps1p", bufs=1, space="PSUM") as pp1,
            tc.tile_pool(name="ps1r", bufs=1, space="PSUM") as pr1,
            tc.tile_pool(name="psw2", bufs=1, space="PSUM") as pw2,
            tc.tile_pool(name="pse2", bufs=2, space="PSUM") as pe2,
        ):
            pc = cpool.tile([N, W2 + 195], bf16)
            nc.sync.dma_start(out=pc[:], in_=pconst_in[:])
            rc = cpool.tile([N, W2 + 130], f32)
            nc.sync.dma_start(out=rc[:], in_=rconst_in[:])
            sg = cpool.tile([N, 2 * NH * CB], bf16)
            nc.sync.dma_start(out=sg[:], in_=sg_in[:])
            wgt = cpool.tile([N, NH], bf16)
            nc.sync.dma_start(out=wgt[:], in_=wgt_in[:])
            qbasis = cpool.tile([3, N], f32)
            nc.sync.dma_start(out=qbasis[:], in_=qbasis_in[:])
            basis2 = cpool.tile([3, N], f32)
            nc.sync.dma_start(out=basis2[:], in_=basis2_in[:])
            ones65 = cpool.tile([N, 1], f32)
            nc.sync.dma_start(out=ones65[:], in_=ones_in[:])
            abg = cpool.tile([3, nwin], f32)
            nc.sync.dma_start(out=abg[:], in_=abg_in[:])
            # f32r copy of [Qr | -Qi] for the 1-cycle stage2 matmuls (the
            # verifier requires f32r operands to be produced as f32r)
            rcr = cpool.tile([N, 130], f32r, tag="rcr")
            nc.vector.tensor_copy(rcr[:], rc[:, W2:W2 + 130])

            strips = cpool.tile([N, RPC * 160], bf16)
            for i in range(RPC):
                nc.sync.dma_start(
                    out=strips[:, i * 160:(i + 1) * 160], in_=nz_in[i:i + N, :]
                )

            # clock warm-up: tiny reads so ACT/DVE observe each setup DMA
            # queue once, capping per-instruction sync waits downstream.
            warm_s = cpool.tile([1, 9], f32, tag="warm_s")
            warm_v = cpool.tile([1, 9], f32, tag="warm_v")
            for idx, src in enumerate(
                (strips, pc, rc, sg, wgt, qbasis, basis2, ones65, abg)
            ):
                nc.scalar.copy(warm_s[0:1, idx:idx + 1], src[0:1, 0:1])
                nc.vector.tensor_copy(warm_v[0:1, idx:idx + 1], src[0:1, 0:1])

            epsp = cpool.tile([N, 1], f32, tag="epsp")
            nc.gpsimd.memset(epsp[:], 1e-8)
            epsn = cpool.tile([N, 1], f32, tag="epsn")
            nc.gpsimd.memset(epsn[:], -1e-8)

            vcs = cpool.tile([N, nwin], f32)
            ss = cpool.tile([N, nwin], f32)
            mus = cpool.tile([1, nwin], f32)

            CWID = CB * N           # 1170 cols per chunk
            for sc0 in range(0, nchunk, SC):
                # ---- phase A: q fields + sqrt (ACT sqrt-table), SC-wide ----
                s_sb = qpool.tile([N, SC * CWID], f32, tag="s_sb")
                for j in range(SC):
                    cw0 = (sc0 + j) * CB
                    rq_sb = rqpool.tile([3, CB * N], f32, tag="rq")
                    rq3 = rq_sb[:].rearrange("p (w c) -> p w c", c=N)
                    abg_v = abg[:, cw0:cw0 + CB].unsqueeze(2).broadcast_to(
                        [3, CB, N]
                    )
                    b2_v = basis2[:].unsqueeze(1).broadcast_to([3, CB, N])
                    nc.vector.tensor_tensor(rq3, abg_v, b2_v, op=ALU.mult)
                    for h in range(3):
                        qps = pq_.tile([N, 390], f32, tag="qps")
                        nc.tensor.matmul(
                            qps[:], qbasis[:], rq_sb[:, h * 390:(h + 1) * 390],
                            start=True, stop=True,
                        )
                        nc.scalar.activation(
                            s_sb[:, j * CWID + h * 390:j * CWID + (h + 1) * 390],
                            qps[:], AF.Sqrt,
                        )
                # ---- phase B: ONE wide exp per superchunk (2 table loads) ----
                r_sb4 = rpool.tile([N, SC * CWID], f32, tag="r_sb")
                nc.scalar.activation(r_sb4[:], s_sb[:], AF.Exp, scale=-1.0)

                # ---- phase C: DFTs + spectral chain per chunk ----
                for j in range(SC):
                    ch = sc0 + j
                    cw0 = ch * CB
                    r_sb = r_sb4[:, j * CWID:(j + 1) * CWID]

                    o1p = o1pool.tile([N, CB * W2], bf16, tag="o1p")
                    o1r = o1pool.tile([N, CB * W2], f32r, tag="o1r")
                    for g in range(NGRP):
                        ps1p = pp1.tile([N, GRP * W2], f32, tag="ps1p")
                        for t in range(GRP):
                            w = cw0 + g * GRP + t
                            r0, c0 = divmod(w, 96)
                            lhs = strips[:, r0 * 160 + c0: r0 * 160 + c0 + N]
                            nc.tensor.matmul(
                                ps1p[:, t * W2:(t + 1) * W2], lhs, pc[:, 0:W2],
                                start=True, stop=True,
                            )
                        nc.scalar.copy(
                            o1p[:, g * GRP * W2:(g + 1) * GRP * W2], ps1p[:]
                        )
                        ps1r = pr1.tile([N, GRP * W2], f32, tag="ps1r")
                        for t in range(GRP):
                            wl = (g * GRP + t) * N
                            nc.tensor.matmul(
                                ps1r[:, t * W2:(t + 1) * W2],
                                r_sb[:, wl:wl + N], rc[:, 0:W2],
                                start=True, stop=True,
                            )
                        nc.vector.tensor_copy(
                            o1r[:, g * GRP * W2:(g + 1) * GRP * W2], ps1r[:]
                        )

                    # batched stage2 + spectral front-end, per 9-window block
                    wfA = wfpool.tile([N, CB * NH], bf16, tag="wfA")
                    wfB = wfpool.tile([N, CB * NH], bf16, tag="wfB")
                    rp = spool.tile([N, CB * NH], bf16, tag="rp")
                    rn = spool.tile([N, CB * NH], bf16, tag="rn")
                    azt = spool.tile([N, CB * NH], bf16, tag="azt")
                    for b in range(2):
                        o1p3 = o1p[:, b * BW * W2:(b + 1) * BW * W2].rearrange(
                            "p (w t) -> p w t", t=W2
                        )
                        o1r3 = o1r[:, b * BW * W2:(b + 1) * BW * W2].rearrange(
                            "p (w t) -> p w t", t=W2
                        )
                        sl = slice(b * BW * NH, (b + 1) * BW * NH)

                        wre = pw2.tile([N, BW * NH], f32, tag="wre")
                        nc.tensor.matmul(
                            wre[:], pc[:, W2:W2 + 65], o1p3[:, :, 0:NH],
                            start=True, stop=False,
                        )
                        nc.tensor.matmul(
                            wre[:], pc[:, W2 + 130:W2 + 195], o1p3[:, :, NH:W2],
                            start=False, stop=True,
                        )
                        nc.scalar.copy(wfA[:, sl], wre[:])
                        wim = pw2.tile([N, BW * NH], f32, tag="wim")
                        nc.tensor.matmul(
                            wim[:], pc[:, W2 + 65:W2 + 130], o1p3[:, :, 0:NH],
                            start=True, stop=False,
                        )
                        nc.tensor.matmul(
                            wim[:], pc[:, W2:W2 + 65], o1p3[:, :, NH:W2],
                            start=False, stop=True,
                        )
                        nc.scalar.copy(wfB[:, sl], wim[:])

                        e2 = pe2.tile([N, BW * NH], f32, tag="e2")
                        nc.tensor.matmul(
                            e2[:], rcr[:, 0:65], o1r3[:, :, 0:NH],
                            start=True, stop=False,
                        )
                        nc.tensor.matmul(
                            e2[:], rcr[:, 65:130], o1r3[:, :, NH:W2],
                            start=False, stop=True,
                        )
                        # rp = relu(E+eps), rn = relu(-(E+eps)), az = |E+eps|
                        nc.scalar.activation(rp[:, sl], e2[:], AF.Relu, bias=epsp[:])
                        nc.scalar.activation(
                            rn[:, sl], e2[:], AF.Relu, scale=-1.0, bias=epsn[:]
                        )

                    FD = CB * NH
                    # az = |E+eps| = relu(E+eps) + relu(-(E+eps))
                    nc.vector.tensor_add(azt[:], rp[:], rn[:])
                    azw = spool.tile([N, FD], bf16, tag="azw")
                    az3o = azw[:].rearrange("p (w c) -> p w c", c=NH)
                    az3i = azt[:].rearrange("p (w c) -> p w c", c=NH)
                    wgt_v = wgt[:].unsqueeze(1).broadcast_to([N, CB, NH])
                    nc.vector.tensor_tensor(az3o, az3i, wgt_v, op=ALU.mult)

                    gp = spool.tile([N, FD], bf16, tag="gp")
                    nc.scalar.activation(gp[:], rp[:], AF.Sqrt)
                    gn = spool.tile([N, FD], bf16, tag="gn")
                    nc.scalar.activation(gn[:], rn[:], AF.Sqrt)
                    gps = spool.tile([N, FD], bf16, tag="gps")
                    nc.vector.tensor_mul(gps[:], gp[:], sg[:, 0:FD])
                    gns = spool.tile([N, FD], bf16, tag="gns")
                    nc.vector.tensor_mul(gns[:], gn[:], sg[:, FD:2 * FD])
                    a_t = spool.tile([N, FD], bf16, tag="a_t")
                    nc.vector.tensor_mul(a_t[:], wfA[:], gps[:])
                    b_t = spool.tile([N, FD], bf16, tag="b_t")
                    nc.vector.tensor_mul(b_t[:], wfB[:], gns[:])
                    xr = spool.tile([N, FD], bf16, tag="xr")
                    nc.vector.tensor_sub(xr[:], a_t[:], b_t[:])
                    wr2 = spool.tile([N, FD], bf16, tag="wr2")
                    nc.vector.tensor_mul(wr2[:], wfA[:], wfA[:])
                    wi2 = spool.tile([N, FD], bf16, tag="wi2")
                    nc.vector.tensor_mul(wi2[:], wfB[:], wfB[:])
                    w2s = spool.tile([N, FD], bf16, tag="w2s")
                    nc.vector.tensor_add(w2s[:], wr2[:], wi2[:])
                    st = spool.tile([N, FD], bf16, tag="st")
                    nc.vector.tensor_mul(st[:], w2s[:], azw[:])

                    xr3 = xr[:].rearrange("p (w c) -> p w c", c=NH)
                    st3 = st[:].rearrange("p (w c) -> p w c", c=NH)
                    nc.vector.tensor_reduce(
                        vcs[:, cw0:cw0 + CB], xr3, axis=AX.X, op=ALU.add
                    )
                    nc.vector.tensor_reduce(
                        ss[:, cw0:cw0 + CB], st3, axis=AX.X, op=ALU.add
                    )
                    nc.vector.tensor_copy(mus[0:1, cw0:cw0 + CB], xr3[0:1, :, 0])

            # ---- on-device finalize: one scalar per window ----
            vc_row = cpool.tile([1, nwin], f32, tag="vc_row")
            s_row = cpool.tile([1, nwin], f32, tag="s_row")
            t1 = cpool.tile([1, nwin], f32, tag="t1")
            t2 = cpool.tile([1, nwin], f32, tag="t2")
            t3 = cpool.tile([1, nwin], f32, tag="t3")
            NB = 384
            for b0 in range(0, nwin, NB):
                bn = min(NB, nwin - b0)
                pv = pq_.tile([1, NB], f32, tag="qps")
                nc.tensor.matmul(
                    pv[:, 0:bn], ones65[:], vcs[:, b0:b0 + bn],
                    start=True, stop=True,
                )
                nc.vector.tensor_copy(vc_row[0:1, b0:b0 + bn], pv[:, 0:bn])
                pv2 = pq_.tile([1, NB], f32, tag="qps")
                nc.tensor.matmul(
                    pv2[:, 0:bn], ones65[:], ss[:, b0:b0 + bn],
                    start=True, stop=True,
                )
                nc.vector.tensor_copy(s_row[0:1, b0:b0 + bn], pv2[:, 0:bn])

            # out = ((VC-mu)/N2) / (sqrt(max(S-mu^2,0)/(N2*(N2-1))) + 1e-6)
            nc.vector.tensor_sub(t1[:], vc_row[:], mus[:])
            nc.vector.tensor_mul(t2[:], mus[:], mus[:])
            nc.vector.tensor_sub(t3[:], s_row[:], t2[:])
            nc.vector.tensor_scalar_max(t2[:], t3[:], 0.0)
            c2 = 1.0 / (float(N2) * (N2 - 1.0))
            nc.scalar.activation(t3[:], t2[:], AF.Sqrt, scale=c2)
            nc.vector.tensor_scalar_add(t2[:], t3[:], 1e-6)
            nc.vector.reciprocal(t3[:], t2[:])
            nc.vector.scalar_tensor_tensor(
                t2[:], t1[:], 1.0 / N2, t3[:], op0=ALU.mult, op1=ALU.mult
            )
            nc.sync.dma_start(out=out_d[:], in_=t2[:])
    if not nc.is_finalized():
        nc.finalize()
    return nc


_HOST_CONSTS = None


def _host_inputs(angle_matrix, noise, nchunk):
    """Per-core input maps. Core c owns output rows [12c, 12c+12)."""
    global _HOST_CONSTS
    if _HOST_CONSTS is None:
        _HOST_CONSTS = _host_constants()
    pconst, rconst, sgconst, wgt33, qbasis, basis2, ones65 = _HOST_CONSTS
    noise2d = np.asarray(noise, dtype=np.float32)[0, 0]
    nzb = noise2d.astype(_bf16)
    ang = np.asarray(angle_matrix, dtype=np.float32).reshape(-1).astype(np.float64)
    c = np.cos(ang); s = np.sin(ang)
    alpha = (c * c / A_**2 + s * s / B_**2).astype(np.float32)
    beta = (s * s / A_**2 + c * c / B_**2).astype(np.float32)
    gamma = (2 * c * s * (1 / A_**2 - 1 / B_**2)).astype(np.float32)
    abg_all = np.stack([alpha, beta, gamma])  # [3, 9216]

    in_maps = []
    for core in range(NCORE):
        w0 = core * WPC
        r_base = core * RPC
        in_maps.append({
            "nz": np.ascontiguousarray(nzb[r_base:r_base + RPC + N - 1, :]),
            "abg": np.ascontiguousarray(abg_all[:, w0:w0 + WPC]),
            "pconst": pconst,
            "rconst": rconst,
            "sgconst": sgconst,
            "wgt33": wgt33,
            "qbasis": qbasis,
            "basis2": basis2,
            "ones65": ones65,
        })
    return in_maps


def _finalize(core_outs):
    """core_outs: list of [1, 1152] arrays -> [96, 96] output."""
    return np.concatenate(
        [arr.reshape(RPC, 96) for arr in core_outs], axis=0
    ).astype(np.float32)


_PROG = {}
_RUNNER = {}
_CONST_NAMES = frozenset(
    {"pconst", "rconst", "sgconst", "wgt33", "qbasis", "basis2", "ones65"}
)
_DEV_CONSTS = {}


def _get_program(nchunk):
    if nchunk not in _PROG:
        _PROG[nchunk] = _build_program(nchunk)
    return _PROG[nchunk]


def _get_runner(nchunk):
    """Build (once) a jitted shard_map executable over the 8 cores.

    Mirrors concourse.bass2jax.run_bass_via_pjrt, but caches the traced/
    compiled callable so repeat kernel() calls skip retracing.
    """
    if nchunk in _RUNNER:
        return _RUNNER[nchunk]
    import jax
    import concourse.mybir as mybir
    from concourse import bass2jax
    from jax.experimental.shard_map import shard_map
    from jax.sharding import Mesh, PartitionSpec

    nc = _get_program(nchunk)
    bass2jax.install_neuronx_cc_hook()
    assert nc.dbg_addr is None
    partition_name = (
        nc.partition_id_tensor.name if nc.partition_id_tensor else None
    )
    in_names, out_names, out_avals, zero_outs = [], [], [], []
    for alloc in nc.m.functions[0].allocations:
        if not isinstance(alloc, mybir.MemoryLocationSet):
            continue
        name = alloc.memorylocations[0].name
        if alloc.kind == "ExternalInput":
            if name != partition_name:
                in_names.append(name)
        elif alloc.kind == "ExternalOutput":
            out_names.append(name)
            shape = tuple(alloc.tensor_shape)
            dtype = mybir.dt.np(alloc.dtype)
            out_avals.append(jax.core.ShapedArray(shape, dtype))
            zero_outs.append(np.zeros(shape, dtype))
    n_params = len(in_names)
    n_outs = len(out_avals)
    in_names_all = in_names + out_names
    if partition_name is not None:
        in_names_all.append(partition_name)
    donate = tuple(range(n_params, n_params + n_outs))

    def _body(*args):
        operands = list(args)
        if partition_name is not None:
            operands.append(bass2jax.partition_id_tensor())
        outs = bass2jax._bass_exec_p.bind(
            *operands,
            out_avals=tuple(out_avals),
            in_names=tuple(in_names_all),
            out_names=tuple(out_names),
            lowering_input_output_aliases=(),
            sim_require_finite=True,
            sim_require_nnan=True,
            nc=nc,
        )
        return tuple(outs)

    devices = jax.devices()[:NCORE]
    mesh = Mesh(np.asarray(devices), ("core",))
    sharded = jax.jit(
        shard_map(
            _body,
            mesh=mesh,
            in_specs=(PartitionSpec("core"),) * (n_params + n_outs),
            out_specs=(PartitionSpec("core"),) * n_outs,
            check_rep=False,
        ),
        donate_argnums=donate,
        keep_unused=True,
    )
    zero_concats = [
        np.zeros((NCORE * z.shape[0], *z.shape[1:]), z.dtype) for z in zero_outs
    ]
    info = (sharded, in_names, out_names, out_avals, zero_concats, mesh)
    _RUNNER[nchunk] = info
    return info


def _run(in_maps, nchunk):
    import jax
    from jax.sharding import NamedSharding, PartitionSpec

    sharded, in_names, out_names, out_avals, zero_concats, mesh = _get_runner(
        nchunk
    )
    args = []
    for name in in_names:
        if name in _CONST_NAMES:
            # constants live on device across calls: zero per-call transfer
            if name not in _DEV_CONSTS:
                concat = np.concatenate(
                    [in_maps[c][name] for c in range(NCORE)], axis=0
                )
                arr = jax.device_put(
                    concat, NamedSharding(mesh, PartitionSpec("core"))
                )
                arr.block_until_ready()
                _DEV_CONSTS[name] = arr
            args.append(_DEV_CONSTS[name])
        else:
            args.append(
                np.concatenate([in_maps[c][name] for c in range(NCORE)], axis=0)
            )
    out_arrs = sharded(*args, *[z.copy() for z in zero_concats])
    outs = {
        name: np.asarray(out_arrs[i]).reshape(NCORE, *out_avals[i].shape)
        for i, name in enumerate(out_names)
    }
    return [outs["out"][c] for c in range(NCORE)]


def kernel(angle_matrix, noise):
    nchunk = WPC // CB
    in_maps = _host_inputs(angle_matrix, noise, nchunk)
    core_outs = _run(in_maps, nchunk)
    return _finalize(core_outs)
